# revision 1
# baseline (speedup 1.0000x reference)
"""Trainium2 Bass kernel for nn_EncoderDecoderCDE.

Model: 2-layer LSTM encoder (T=50) -> linear+tanh map to CDE state (64)
-> 16 fixed dopri5 steps x 6 stages of a neural-CDE vector field
(LayerNorm -> Linear 64->128 -> ReLU -> Linear 128->2112 -> tanh ->
einsum with spline derivative) -> final projection to 10000 zones.

Distribution: pure data parallel, batch 1024 = 8 cores x 128.
Per-core layout choices:
  * LSTM runs feature-on-partition ("T layout"): weights are the matmul
    stationary operand, gate order reordered to (i, f, o, g) so one
    sigmoid ACT call covers i,f,o.  Layer-0 bias is folded in via an
    appended ones-row on the input (k=97); layer-1 bias via k=1 rank-1
    matmuls.
  * CDE state z is batch-on-partition [128, 64].  LayerNorm stats come
    from STT accumulators + an ACT Square; rstd is computed with Newton
    iterations on DVE (no sqrt on ACT: tanh/sigmoid and sqrt never share
    an ACT table set, so per-stage sqrt would thrash table loads).
    rstd is factored out of the MLP:  relu(rstd*X + c) = rstd*relu(X + std*c)
    and the trailing rstd is applied as the per-partition ACT scale of the
    final tanh; the std*c / std*b2 terms enter PSUM as k=1 rank-1 matmuls
    whose lhsT is the std row produced by the same PE transpose that
    transposes zc.
  * The einsum  out[b,h] = sum_i tanh(f)[b,h*33+i] * dX[b,i]  is one DVE
    tensor_tensor multiply with a step-0-broadcast AP on dX, followed by
    a log-tree of strided adds over i (33 = 32 + 1).
All matmuls run in bf16 (fp32 matmul is 4x slower on the PE).
"""

import math
import os

import numpy as np
import ml_dtypes

B = 1024
T_HIST = 50
ENC_IN = 96
ENC_H = 128
CDE_H = 64
CH = 33
NZ = 10000
NI = 4
MLP = 128
N_STEPS = 16
N_CORES = 8
BC = B // N_CORES  # 128 batch per core
F2 = CDE_H * CH  # 2112

# dopri5 tableau (from the reference implementation)
DOPRI_C = [0.0, 1 / 5, 3 / 10, 4 / 5, 8 / 9, 1.0]
DOPRI_A = [
    [],
    [1 / 5],
    [3 / 40, 9 / 40],
    [44 / 45, -56 / 15, 32 / 9],
    [19372 / 6561, -25360 / 2187, 64448 / 6561, -212 / 729],
    [9017 / 3168, -355 / 33, 46732 / 5247, 49 / 176, -5103 / 18656],
]
DOPRI_B = [35 / 384, 0.0, 500 / 1113, 125 / 192, -2187 / 6784, 11 / 84]

BF16 = ml_dtypes.bfloat16

GATE_ORDER = [0, 1, 3, 2]  # torch (i,f,g,o) -> ours (i,f,o,g)


def _reorder_gates(w):
    """w: (4*H, ...) in torch gate order i,f,g,o -> (4*H, ...) as i,f,o,g."""
    chunks = np.split(w, 4, axis=0)
    return np.concatenate([chunks[g] for g in GATE_ORDER], axis=0)


def _pack_host(inputs):
    """Shard + transpose + cast everything on the host. Returns a list of
    per-core input dicts."""
    f32 = np.float32
    hp = np.asarray(inputs["history_path"], f32)          # (B, 50, 96)
    cc = np.asarray(inputs["cde_coeffs"], f32)            # (B, 4, 4, 33)

    W_ih0 = _reorder_gates(np.asarray(inputs["W_ih0"], f32))
    W_hh0 = _reorder_gates(np.asarray(inputs["W_hh0"], f32))
    b0 = _reorder_gates((np.asarray(inputs["b_ih0"], f32)
                         + np.asarray(inputs["b_hh0"], f32))[:, None])[:, 0]
    W_ih1 = _reorder_gates(np.asarray(inputs["W_ih1"], f32))
    W_hh1 = _reorder_gates(np.asarray(inputs["W_hh1"], f32))
    b1l = _reorder_gates((np.asarray(inputs["b_ih1"], f32)
                          + np.asarray(inputs["b_hh1"], f32))[:, None])[:, 0]

    W_map = np.asarray(inputs["W_map"], f32)              # (64, 128)
    b_map = np.asarray(inputs["b_map"], f32)              # (64,)
    gamma = np.asarray(inputs["gamma"], f32)              # (64,)
    beta = np.asarray(inputs["beta"], f32)                # (64,)
    W1 = np.asarray(inputs["W1"], f32)                    # (128, 64)
    b1 = np.asarray(inputs["b1"], f32)                    # (128,)
    W2 = np.asarray(inputs["W2"], f32)                    # (2112, 128)
    b2 = np.asarray(inputs["b2"], f32)                    # (2112,)
    W_pred = np.asarray(inputs["W_pred"], f32)            # (10000, 64)
    b_pred = np.asarray(inputs["b_pred"], f32)            # (10000,)

    shared = {}
    shared["wih0"] = np.ascontiguousarray(np.concatenate(
        [W_ih0.T, b0[None, :]], axis=0)).astype(BF16)     # (97, 512)
    shared["whh0"] = np.ascontiguousarray(W_hh0.T).astype(BF16)
    shared["wih1"] = np.ascontiguousarray(W_ih1.T).astype(BF16)
    shared["whh1"] = np.ascontiguousarray(W_hh1.T).astype(BF16)
    shared["bias1"] = b1l[None, :].astype(BF16)           # (1, 512)
    shared["wmap"] = np.ascontiguousarray(W_map.T).astype(BF16)  # (128, 64)
    shared["bmap"] = b_map[None, :].astype(BF16)          # (1, 64)
    W1g = W1 * gamma[None, :]                             # fold gamma
    shared["w1t"] = np.ascontiguousarray(W1g.T).astype(BF16)     # (64, 128)
    c_vec = W1 @ beta + b1                                # fold beta
    shared["crow"] = c_vec[None, :].astype(BF16)          # (1, 128)
    shared["w2t"] = np.ascontiguousarray(W2.T).astype(BF16)      # (128, 2112)
    shared["b2row"] = b2[None, :].astype(BF16)            # (1, 2112)
    shared["wpt"] = np.ascontiguousarray(W_pred.T).astype(BF16)  # (64, 10000)
    shared["bprow"] = b_pred[None, :].astype(BF16)        # (1, 10000)
    shared["ident"] = np.eye(128, dtype=BF16)             # (128, 128)

    per_core = []
    t_hist = hp.shape[1]
    for c in range(N_CORES):
        sl = slice(c * BC, (c + 1) * BC)
        h = hp[sl]                                        # (128, T, 96)
        ht = np.ascontiguousarray(h.transpose(2, 1, 0))   # (96, T, 128)
        ht = ht.reshape(ENC_IN, t_hist * BC)
        ht_aug = np.concatenate(
            [ht, np.ones((1, t_hist * BC), f32)], axis=0)  # (97, T*128)
        d = dict(shared)
        d["hist"] = ht_aug.astype(BF16)
        d["cc"] = np.ascontiguousarray(cc[sl].reshape(BC, NI * 4 * CH))
        d["chain"] = np.zeros((1, 1), f32)
        per_core.append(d)
    return per_core


def _spline_eval_points(n_steps):
    """(idx, u) for each of the 6*n_steps vf evaluations. t0=0, t1=1."""
    t0, t1 = 0.0, 1.0
    seg = (t1 - t0) / NI
    dt = (t1 - t0) / n_steps
    pts = []
    for i in range(n_steps):
        for s in range(6):
            t = t0 + i * dt + DOPRI_C[s] * dt
            idx = int(np.clip(math.floor((t - t0) / seg), 0, NI - 1))
            u = t - (t0 + idx * seg)
            pts.append((idx, u))
    return pts


def _split_waits(nc, mybir, limit=1):
    """The neuronx-cc walrus build allows at most one sync-wait per
    instruction; hoist extras onto preceding single-wait EventSemaphore
    instructions on the same engine."""
    ctr = 0
    for f in nc.m.functions:
        for bb in f.blocks:
            new = []
            for ins in bb.instructions:
                si = ins.sync_info
                if si is not None and len(si.on_wait) > limit:
                    extra = list(si.on_wait[:-limit])
                    keep = list(si.on_wait[-limit:])
                    for j, w in enumerate(extra):
                        ctr += 1
                        new.append(mybir.InstEventSemaphore(
                            name=f"{ins.name}-ws{j}",
                            engine=ins.engine,
                            sync_info=mybir.SyncInfo(
                                on_wait=[w], on_update=[]),
                        ))
                    ins.sync_info = mybir.SyncInfo(
                        on_wait=keep, on_update=list(si.on_update))
                new.append(ins)
            bb.instructions[:] = new
    return ctr


def build_bass(n_steps=N_STEPS, lstm_steps=T_HIST, split_waits=True,
               repeat=1):
    import concourse.bass as bass
    import concourse.tile as tile
    from concourse import mybir
    from contextlib import ExitStack

    dt_f32 = mybir.dt.float32
    dt_bf16 = mybir.dt.bfloat16
    dt_i32 = mybir.dt.int32
    AF = mybir.ActivationFunctionType
    OP = mybir.AluOpType

    nc = bass.Bass("TRN2", target_bir_lowering=False, debug=False)
    dbg = {}
    build_bass.dbg = dbg

    # ---- I/O -----------------------------------------------------------
    def din(name, shape, dt):
        return nc.dram_tensor(name, shape, dt, kind="ExternalInput").ap()

    hist_d = din("hist", [ENC_IN + 1, lstm_steps * BC], dt_bf16)
    cc_d = din("cc", [BC, NI * 4 * CH], dt_f32)
    wih0_d = din("wih0", [ENC_IN + 1, 4 * ENC_H], dt_bf16)
    whh0_d = din("whh0", [ENC_H, 4 * ENC_H], dt_bf16)
    wih1_d = din("wih1", [ENC_H, 4 * ENC_H], dt_bf16)
    whh1_d = din("whh1", [ENC_H, 4 * ENC_H], dt_bf16)
    bias1_d = din("bias1", [1, 4 * ENC_H], dt_bf16)
    wmap_d = din("wmap", [ENC_H, CDE_H], dt_bf16)
    bmap_d = din("bmap", [1, CDE_H], dt_bf16)
    w1t_d = din("w1t", [CDE_H, MLP], dt_bf16)
    crow_d = din("crow", [1, MLP], dt_bf16)
    w2t_d = din("w2t", [MLP, F2], dt_bf16)
    b2row_d = din("b2row", [1, F2], dt_bf16)
    wpt_d = din("wpt", [CDE_H, NZ], dt_bf16)
    bprow_d = din("bprow", [1, NZ], dt_bf16)
    ident_d = din("ident", [128, 128], dt_bf16)
    chain_d = din("chain", [1, 1], dt_f32)
    out_d = nc.dram_tensor("out", [BC, NZ], dt_f32, kind="ExternalOutput").ap()
    tick_d = nc.dram_tensor("tick", [1, 1], dt_f32, kind="ExternalOutput").ap()

    spline_pts = _spline_eval_points(n_steps)
    dt_step = 1.0 / n_steps

    with tile.TileContext(nc) as tc, ExitStack() as ctx:
        const = ctx.enter_context(tc.tile_pool(name="const", bufs=1))

        # --- load constants -------------------------------------------
        def load(name, ap_d, shape, dt):
            t = const.tile(shape, dt, tag=f"c_{name}")
            nc.sync.dma_start(out=t, in_=ap_d)
            return t

        hist = load("hist", hist_d, [ENC_IN + 1, lstm_steps * BC], dt_bf16)
        cc = load("cc", cc_d, [BC, NI * 4 * CH], dt_f32)
        wih0 = load("wih0", wih0_d, [ENC_IN + 1, 4 * ENC_H], dt_bf16)
        whh0 = load("whh0", whh0_d, [ENC_H, 4 * ENC_H], dt_bf16)
        wih1 = load("wih1", wih1_d, [ENC_H, 4 * ENC_H], dt_bf16)
        whh1 = load("whh1", whh1_d, [ENC_H, 4 * ENC_H], dt_bf16)
        bias1 = load("bias1", bias1_d, [1, 4 * ENC_H], dt_bf16)
        wmap = load("wmap", wmap_d, [ENC_H, CDE_H], dt_bf16)
        bmap = load("bmap", bmap_d, [1, CDE_H], dt_bf16)
        w1t = load("w1t", w1t_d, [CDE_H, MLP], dt_bf16)
        crow = load("crow", crow_d, [1, MLP], dt_bf16)
        w2t = load("w2t", w2t_d, [MLP, F2], dt_bf16)
        b2row = load("b2row", b2row_d, [1, F2], dt_bf16)
        wpt = load("wpt", wpt_d, [CDE_H, NZ], dt_bf16)
        bprow = load("bprow", bprow_d, [1, NZ], dt_bf16)
        ident = load("ident", ident_d, [128, 128], dt_bf16)

        ones_row = const.tile([1, 128], dt_bf16)
        nc.vector.memset(ones_row, 1.0)
        zeros_row = const.tile([1, 128], dt_bf16)
        nc.vector.memset(zeros_row, 0.0)
        eps_t = const.tile([128, 1], dt_f32, tag="eps_t")
        nc.vector.memset(eps_t, 1e-5)
        # benchmark plumbing: consume "chain", produce "tick" (pure
        # dataflow serialization for chained timing runs)
        chain_sb = const.tile([1, 1], dt_f32, tag="chain_sb")
        nc.sync.dma_start(out=chain_sb, in_=chain_d)
        nc.sync.dma_start(out=tick_d, in_=chain_sb)

        rep_pool = ctx.enter_context(tc.tile_pool(name="rep", bufs=2))
        for _rep in range(repeat):
            if _rep > 0:
                # serialize benchmark repeats: gate this iteration's first
                # matmuls on the previous iteration's output DMA
                prev_row = rep_pool.tile([1, 128], dt_bf16, tag="prev_row")
                nc.sync.dma_start(out=prev_row,
                                  in_=out_d[0:1, 0:128])
            # =============================================================
            # Phase 1: 2-layer LSTM encoder, T layout (features on partitions)
            # =============================================================
            lstm_sb = ctx.enter_context(tc.tile_pool(name="lstm_sb", bufs=3))
            lstm_state = ctx.enter_context(tc.tile_pool(name="lstm_state", bufs=2))

            h0T = None  # [128, 128] bf16, layer-0 hidden (features x batch)
            c0 = None   # [128, 128] f32
            h1T = None
            c1 = None

            def lstm_cell(psum_g, c_prev, tag):
                """psum_g: [128, 4, 128] accumulated gates (i,f,o,g).
                Returns (hT bf16, c f32)."""
                sig = lstm_sb.tile([128, 3, 128], dt_bf16, tag=f"sig{tag}")
                nc.scalar.activation(sig, psum_g[:, 0:3, :], AF.Sigmoid)
                gg = lstm_sb.tile([128, 128], dt_bf16, tag=f"gg{tag}")
                nc.scalar.activation(gg, psum_g[:, 3, :], AF.Tanh)
                ig = lstm_sb.tile([128, 128], dt_bf16, tag=f"ig{tag}")
                nc.vector.tensor_mul(ig, sig[:, 0, :], gg)
                c_new = lstm_state.tile([128, 128], dt_f32, tag=f"c{tag}")
                if c_prev is None:
                    nc.vector.tensor_copy(c_new, ig)  # c0 = 0 -> c' = i*g
                else:
                    fc = lstm_sb.tile([128, 128], dt_f32, tag=f"fc{tag}")
                    nc.vector.tensor_mul(fc, sig[:, 1, :], c_prev)
                    nc.vector.tensor_add(c_new, fc, ig)
                thc = lstm_sb.tile([128, 128], dt_bf16, tag=f"thc{tag}")
                nc.scalar.activation(thc, c_new, AF.Tanh)
                h_new = lstm_state.tile([128, 128], dt_bf16, tag=f"h{tag}")
                nc.vector.tensor_mul(h_new, sig[:, 2, :], thc)
                return h_new, c_new

            with tc.tile_pool(name="lstm_psum", bufs=2, space="PSUM") as lstm_psum:
                for t in range(lstm_steps):
                    # ---- layer 0 ----
                    pg0 = lstm_psum.tile([128, 4, 128], dt_f32, tag="pg0")
                    xT = hist[:, t * BC:(t + 1) * BC]  # [97, 128]
                    for g in range(4):
                        gs = slice(g * 128, (g + 1) * 128)
                        nc.tensor.matmul(pg0[:, g, :], wih0[:, gs], xT,
                                         start=True, stop=(t == 0))
                        if t > 0:
                            nc.tensor.matmul(pg0[:, g, :], whh0[:, gs], h0T,
                                             start=False, stop=True)
                    h0T, c0 = lstm_cell(pg0, c0, "L0")

                    # ---- layer 1 ----
                    pg1 = lstm_psum.tile([128, 4, 128], dt_f32, tag="pg1")
                    for g in range(4):
                        gs = slice(g * 128, (g + 1) * 128)
                        nc.tensor.matmul(pg1[:, g, :], bias1[:, gs],
                                         ones_row, start=True, stop=False)
                        nc.tensor.matmul(pg1[:, g, :], wih1[:, gs], h0T,
                                         start=False, stop=(t == 0))
                        if t > 0:
                            nc.tensor.matmul(pg1[:, g, :], whh1[:, gs], h1T,
                                             start=False, stop=True)
                    h1T, c1 = lstm_cell(pg1, c1, "L1")

            # =============================================================
            # Phase 2: CDE solve
            # =============================================================
            wide = ctx.enter_context(tc.tile_pool(name="wide", bufs=2))
            tiny = ctx.enter_context(tc.tile_pool(name="tiny", bufs=6))
            zpool = ctx.enter_context(tc.tile_pool(name="zpool", bufs=2))
            zacc = ctx.enter_context(tc.tile_pool(name="zacc", bufs=5))
            kpool = ctx.enter_context(tc.tile_pool(name="kpool", bufs=7))
            psum_s = ctx.enter_context(
                tc.tile_pool(name="psum_s", bufs=1, space="PSUM"))

            def ln_u(sum_z_t, sum_z2_t):
                """u = var + eps = sum_z2/64 - (mu^2 - eps); also returns mu."""
                mu = tiny.tile([BC, 1], dt_f32, tag="mu")
                nc.vector.tensor_scalar(mu, sum_z_t, 1.0 / CDE_H, None, OP.mult)
                m2e = tiny.tile([BC, 1], dt_f32, tag="m2e")
                nc.vector.scalar_tensor_tensor(m2e, mu, mu, eps_t,
                                               OP.mult, OP.subtract)
                u = tiny.tile([BC, 1], dt_f32, tag="u")
                nc.vector.scalar_tensor_tensor(u, sum_z2_t, 1.0 / CDE_H, m2e,
                                               OP.mult, OP.subtract)
                return mu, u

            def rsqrt(u, n, final_tag):
                sh = tiny.tile([BC, 1], dt_i32, tag="seed_sh")
                nc.vector.tensor_scalar(sh, u.bitcast(dt_i32), 1, None,
                                        OP.logical_shift_right)
                notsh = tiny.tile([BC, 1], dt_i32, tag="seed_not")
                nc.vector.tensor_scalar(notsh, sh, -1, None, OP.bitwise_xor)
                seed = tiny.tile([BC, 1], dt_f32, tag="seed_f")
                nc.vector.tensor_scalar(seed.bitcast(dt_i32), notsh,
                                        0x5F3759E0, None, OP.add)
                y = seed
                for it in range(n):
                    ysq = tiny.tile([BC, 1], dt_f32, tag="n_ysq")
                    nc.vector.tensor_scalar(ysq, y, y, None, OP.mult)
                    h3n = tiny.tile([BC, 1], dt_f32, tag="n_h3n")
                    nc.vector.tensor_scalar(h3n, ysq, y, -0.5, OP.mult, OP.mult)
                    y15 = tiny.tile([BC, 1], dt_f32, tag="n_y15")
                    nc.vector.tensor_scalar(y15, y, 1.5, None, OP.mult)
                    y2 = tiny.tile([BC, 1], dt_f32,
                                   tag=(final_tag if it == n - 1 else "n_y"))
                    nc.vector.scalar_tensor_tensor(y2, u, h3n, y15,
                                                   OP.mult, OP.add)
                    y = y2
                return y

            with ExitStack() as cde_ctx:
                psum_w = cde_ctx.enter_context(
                    tc.tile_pool(name="psum_w", bufs=1, space="PSUM"))

                # ---- z0 = tanh(W_map @ h1 + b_map), [128b, 64] ----------
                pz0 = psum_s.tile([BC, MLP], dt_f32, tag="pa")
                nc.tensor.matmul(pz0[:, 0:CDE_H], ones_row, bmap,
                                 start=True, stop=False)
                nc.tensor.matmul(pz0[:, 0:CDE_H], h1T, wmap,
                                 start=False, stop=True)

                z = zpool.tile([BC, CDE_H], dt_f32, tag="z")
                sum_z = tiny.tile([BC, 1], dt_f32, tag="sum_z")
                nc.scalar.activation(z, pz0[:, 0:CDE_H], AF.Tanh,
                                     accum_out=sum_z)

                sq_scr = wide.tile([BC, CDE_H], dt_f32, tag="sq_scr")
                sum_z2 = tiny.tile([BC, 1], dt_f32, tag="sum_z2")
                nc.scalar.activation(sq_scr, z, AF.Square, accum_out=sum_z2)

                mu0, u0 = ln_u(sum_z, sum_z2)
                dbg["z0"] = z
                dbg["u0"] = u0
                dbg["h1T"] = h1T

                def vf_stage(z_s, sum_z_s, idx, u_sp, first):
                    """One vector-field evaluation at state z_s.  Returns
                    (k tile [128,64] f32, rstd tile)."""
                    # --- dX = cc_b + 2u*cc_c + 3u^2*cc_d (Horner) --------
                    base = idx * (4 * CH)
                    ccb = cc[:, base + CH: base + 2 * CH]
                    ccc = cc[:, base + 2 * CH: base + 3 * CH]
                    ccd = cc[:, base + 3 * CH: base + 4 * CH]
                    t1 = tiny.tile([BC, CH], dt_f32, tag="dx_t1")
                    nc.vector.scalar_tensor_tensor(t1, ccd, 1.5 * u_sp, ccc,
                                                   OP.mult, OP.add)
                    dx = tiny.tile([BC, CH], dt_bf16, tag="dx")
                    nc.vector.scalar_tensor_tensor(dx, t1, 2.0 * u_sp, ccb,
                                                   OP.mult, OP.add)

                    # --- LN stats ----------------------------------------
                    if first:
                        mu, u = mu0, u0
                    else:
                        sq = wide.tile([BC, CDE_H], dt_f32, tag="sq_scr")
                        sum2 = tiny.tile([BC, 1], dt_f32, tag="sum_z2")
                        nc.scalar.activation(sq, z_s, AF.Square, accum_out=sum2)
                        mu, u = ln_u(sum_z_s, sum2)

                    rstd = rsqrt(u, 2 if first else 1, "rstd")
                    dbg.setdefault("stage_u", []).append(u)
                    dbg.setdefault("stage_rstd", []).append(rstd)
                    mb = tiny.tile([BC, 1], dt_f32, tag="mb")
                    nc.vector.tensor_scalar(mb, mu, rstd, -1.0,
                                            OP.mult, OP.mult)

                    # --- zn = rstd*z - mu*rstd (one ACT affine), transpose
                    zn = wide.tile([BC, CDE_H], dt_bf16, tag="zn")
                    nc.scalar.activation(zn, z_s, AF.Identity,
                                         bias=mb, scale=rstd)
                    pT = psum_s.tile([CDE_H, BC], dt_bf16, tag="pT")
                    nc.tensor.transpose(pT, zn, ident)
                    zT = wide.tile([CDE_H, BC], dt_bf16, tag="zT")
                    nc.vector.tensor_copy(zT, pT)

                    # --- mm1 + relu:  aT = relu(W1g znT + c) -------------
                    pa = psum_s.tile([MLP, BC], dt_f32, tag="pa")
                    nc.tensor.matmul(pa, crow, ones_row,
                                     start=True, stop=False)
                    nc.tensor.matmul(pa, w1t, zT,
                                     start=False, stop=True)
                    aT = wide.tile([MLP, BC], dt_bf16, tag="aT")
                    nc.scalar.activation(aT, pa, AF.Relu)

                    # --- 2-chunk pipelined mm2 -> tanh -> mul -> reduce --
                    k = kpool.tile([BC, CDE_H], dt_f32, tag="k")
                    HCH = CDE_H // 2  # 32 h per chunk
                    NCH = HCH * CH    # 1056
                    for c in range(2):
                        off = c * NCH
                        pf = psum_w.tile([BC, NCH], dt_f32, tag="pf")
                        sub = [(0, 512), (512, 512), (1024, NCH - 1024)]
                        for so, sn in sub:
                            nc.tensor.matmul(pf[:, so:so + sn], ones_row,
                                             b2row[:, off + so:off + so + sn],
                                             start=True, stop=False)
                        for so, sn in sub:
                            nc.tensor.matmul(pf[:, so:so + sn], aT,
                                             w2t[:, off + so:off + so + sn],
                                             start=False, stop=True)
                        f_sb = wide.tile([BC, HCH, CH], dt_bf16, tag="f_sb")
                        nc.scalar.activation(
                            f_sb.rearrange("p a b -> p (a b)"), pf, AF.Tanh)
                        prod = wide.tile([BC, HCH, CH], dt_bf16, tag="prod")
                        dx_b = dx.unsqueeze(1).broadcast_to([BC, HCH, CH])
                        nc.vector.tensor_mul(prod, f_sb, dx_b)
                        nc.vector.tensor_reduce(k[:, c * HCH:(c + 1) * HCH],
                                                prod, mybir.AxisListType.X,
                                                OP.add)
                        dbg.setdefault("stage_f", []).append(f_sb)
                    dbg.setdefault("stage_k", []).append(k)
                    dbg.setdefault("stage_dx", []).append(dx)
                    return k, rstd

                for istep in range(n_steps):
                    ks = []
                    z_s, sum_z_s = z, sum_z
                    for s in range(6):
                        idx, u_sp = spline_pts[istep * 6 + s]
                        k, _ = vf_stage(z_s, sum_z_s, idx, u_sp,
                                        first=(istep == 0 and s == 0))
                        ks.append(k)
                        # build the next stage input (or the step update)
                        if s < 5:
                            coeffs = [dt_step * a for a in DOPRI_A[s + 1]]
                        else:
                            coeffs = [dt_step * b for b in DOPRI_B]
                        acc = z
                        n_terms = sum(1 for cf in coeffs if cf != 0.0)
                        seen = 0
                        s_acc = None
                        for j, cf in enumerate(coeffs):
                            if cf == 0.0:
                                continue
                            seen += 1
                            last = seen == n_terms
                            if last and s == 5:
                                nz_new = zpool.tile([BC, CDE_H], dt_f32, tag="z")
                            else:
                                nz_new = zacc.tile([BC, CDE_H], dt_f32,
                                                   tag="z_acc")
                            if last:
                                s_acc = tiny.tile([BC, 1], dt_f32, tag="sum_z")
                                nc.vector.scalar_tensor_tensor(
                                    nz_new, ks[j], cf, acc, OP.mult, OP.add,
                                    accum_out=s_acc)
                            else:
                                nc.vector.scalar_tensor_tensor(
                                    nz_new, ks[j], cf, acc, OP.mult, OP.add)
                            acc = nz_new
                        if s < 5:
                            z_s, sum_z_s = acc, s_acc
                        else:
                            z, sum_z = acc, s_acc

            # =============================================================
            # Phase 3: out = z @ W_pred.T + b_pred
            # =============================================================
            zb = wide.tile([BC, CDE_H], dt_bf16, tag="zb")
            nc.vector.tensor_copy(zb, z)
            pzT = psum_s.tile([CDE_H, BC], dt_bf16, tag="pT")
            nc.tensor.transpose(pzT, zb, ident)
            zT_f = wide.tile([CDE_H, BC], dt_bf16, tag="zT_f")
            nc.vector.tensor_copy(zT_f, pzT)

            with tc.tile_pool(name="out_psum", bufs=3, space="PSUM") as out_psum, \
                    tc.tile_pool(name="out_sb", bufs=3) as out_sb:
                for off in range(0, NZ, 512):
                    n = min(512, NZ - off)
                    po = out_psum.tile([BC, 512], dt_f32, tag="po")
                    nc.tensor.matmul(po[:, 0:n], ones_row,
                                     bprow[:, off:off + n],
                                     start=True, stop=False)
                    nc.tensor.matmul(po[:, 0:n], zT_f, wpt[:, off:off + n],
                                     start=False, stop=True)
                    ob = out_sb.tile([BC, 512], dt_f32, tag="ob")
                    nc.scalar.activation(ob[:, 0:n], po[:, 0:n], AF.Copy)
                    nc.sync.dma_start(out=out_d[:, off:off + n], in_=ob[:, 0:n])


    if split_waits:
        build_bass.n_split = _split_waits(nc, mybir, 1)
    return nc


_CACHE = {}


def _get_bass(n_steps=N_STEPS):
    if n_steps not in _CACHE:
        _CACHE[n_steps] = build_bass(n_steps)
    return _CACHE[n_steps]


def kernel(**inputs):
    from concourse.bass_utils import run_bass_kernel_spmd

    nc = _get_bass()
    per_core = _pack_host(inputs)
    res = run_bass_kernel_spmd(
        nc, per_core, core_ids=list(range(N_CORES)),
        trace=bool(int(os.environ.get("KERNEL_TRACE", "0"))))
    out = np.concatenate([res.results[c]["out"] for c in range(N_CORES)],
                         axis=0)
    kernel.last_results = res
    return out.astype(np.float32)

